# revision 48
# baseline (speedup 1.0000x reference)
# Trainium2 Bass kernel for NonLocalBlock (B=4, C=64, CI=32, H=W=80).
#
# Math (per batch, N = H*W = 6400):
#   u = Wu@x+bu, v = Wv@x+bv, g = Wg@x+bg           [CI, N]
#   f[n,m] = sum_c u[c,n] v[c,m]; softmax over n (axis=1 of f)
#   y[c,n] = sum_m f_sm[n,m] g[c,m];  out = Ww@y + bw + x
#
# Define S = v^T u  (S[m,n] = f[n,m]).  The softmax axis n is the FREE
# axis of S rows, so processing S in 128-row blocks makes the softmax
# fully row-local.  y = g @ softmax_rows(S).
#
# Sharding: 8 cores = 4 batches x 2 halves of the m axis.  Each core
# computes a partial y (sum over its 3200 m rows), applies the output
# projection, and the host adds the two halves (bias+residual carried
# by the odd core via `resid`; even cores get zeros).
#
# Exp is split across engines: ACT chunks {0,2,4,6} of each 128x6400
# block run the exact Exp activation (rowsums via fused accum); DVE
# chunks {1,3,5} use the Schraudolph bit trick - the host pre-scales
# the matching x_aug columns by log2(e) so the S matmul directly
# yields t = S*log2e, and one DVE tensor_scalar computes
# i16 = round(t*128 + 16248.75), whose bits reinterpreted as bf16 are
# 2^t = e^S with ~1.8% rms multiplicative error (zero mean).  Softmax
# normalization + averaging over m wash this to ~1e-3 in the output.
# DVE-chunk rowsums come from bf16 tensor_reduce on DVE/GPSIMD.
#
# Engine budget per core/block (steady state): PE ~5.4-7us (13 S
# matmuls + 13 y matmuls, fp16/bf16 at 1 col/cycle), ACT ~5.0us,
# DVE ~4.6us, GPSIMD ~3.5us.  PE is the roofline.

import numpy as np

import concourse.bass as bass
import concourse.mybir as mybir
from concourse import bacc, tile
from concourse.bass_utils import run_bass_kernel_spmd

F32 = mybir.dt.float32
BF16 = mybir.dt.bfloat16
F16 = mybir.dt.float16
I16 = mybir.dt.int16

B, C, CI, H, W = 4, 64, 32, 80, 80
N = H * W              # 6400
NCORES = 8
MH = N // 2            # 3200 rows of S per core
MB = 128               # S row-block
NBLK = MH // MB        # 25 blocks per core
SCH = 1024             # S free-dim chunk held in PSUM (2 banks)
YCH = 512              # y free-dim chunk (1 bank)

EXP = mybir.ActivationFunctionType.Exp
ADD = mybir.AluOpType.add
MULT = mybir.AluOpType.mult
AXLX = mybir.AxisListType.X

LOG2E = 1.4426950408889634
C_SCHRAUD = 16248.75   # 127*128 - 128*log2(mean mult err), zero-mean tuned

# chunk assignment within each block (indices into S_CHUNKS); sized by
# measured rates: ACT ~1.37 ns/elem (exp+accum), DVE ~1.97 ns/elem
# (schraudolph + separate rowsum reduce).  The two big DVE chunks are
# ADJACENT so one contiguous 2048-wide reduce covers both rowsums.
DVE_CHUNKS = (1, 2, 6)          # schraudolph on DVE (host scales these cols)
ACT_SCOL = {0: 0, 3: 1, 4: 2, 5: 3}   # sums column per ACT chunk


def _ceil_chunks(total, step):
    out = []
    off = 0
    while off < total:
        out.append((off, min(step, total - off)))
        off += step
    return out


S_CHUNKS = _ceil_chunks(N, SCH)      # 6 x 1024 + 256
Y_CHUNKS = _ceil_chunks(N, YCH)      # 12 x 512 + 256
U_CHUNKS = _ceil_chunks(N, 512)      # 12 x 512 + 256
V_CHUNKS = _ceil_chunks(MH, 512)     # 6 x 512 + 128
NSC = len(S_CHUNKS)                  # 7
# y chunks interleaved into the S chunk stream: after S chunk ci of
# block i, emit y chunks Y_SCHED[ci] of block i-1.  Quads use all four
# PE column groups so the matmuls can pipeline on disjoint tiles.
Y_SCHED = [(0, 1), (2, 3), (4, 5), (6, 7), (8, 9), (10, 11), (12,)]


def build_nc():
    nc = bacc.Bacc("TRN2", target_bir_lowering=False, debug=False,
                   num_devices=NCORES)

    x_aug_d = nc.dram_tensor("x_aug", [C + 1, N], F16, kind="ExternalInput")
    x_m_d = nc.dram_tensor("x_m", [C + 1, MH], F16, kind="ExternalInput")
    w3_d = nc.dram_tensor("w3", [C + 1, 3 * CI], F16, kind="ExternalInput")
    wwT4_d = nc.dram_tensor("wwT4", [128, C], F16, kind="ExternalInput")
    resid_d = nc.dram_tensor("resid", [C, N], F32, kind="ExternalInput")
    out_d = nc.dram_tensor("out", [C, N], F32, kind="ExternalOutput")

    with tile.TileContext(nc) as tc:
        with (
            tc.tile_pool(name="const", bufs=1) as cpool,
            tc.tile_pool(name="big", bufs=2) as dpool,
            tc.tile_pool(name="small", bufs=3) as wpool,
            tc.tile_pool(name="ypsum", bufs=1, space="PSUM") as ypool,
        ):
            # ---- persistent SBUF tiles ----
            x_aug = cpool.tile([C + 1, N], F16, tag="xa")
            x_m = cpool.tile([C + 1, MH], F16, tag="xm")
            u_sb = cpool.tile([2 * CI, N], F16, tag="u")     # 2 row groups
            v_sb = cpool.tile([2 * CI, MH], F16, tag="v")
            gt_sb = cpool.tile([128, NBLK * CI], F32, tag="gt")
            w3 = cpool.tile([C + 1, 3 * CI], F16, tag="w3")
            wwT4 = cpool.tile([128, C], F16, tag="ww")
            resid = cpool.tile([C, N], F32, tag="resid")
            # tail staging: one [128, 512] image per y PSUM bank (each bank
            # holds four y chunks at partition offsets 0/32/64/96)
            y_sbs = [cpool.tile([128, YCH], F16, tag=f"ysb{t}",
                                name=f"ysb{t}") for t in range(4)]
            wuT = w3[:, 0:CI]
            wvT = w3[:, CI:2 * CI]
            wgT = w3[:, 2 * CI:3 * CI]

            # ---- input DMAs: sync carries the x tensors (ordered so the
            # first projections can start asap), gpsimd the small weights
            nc.gpsimd.dma_start(w3[:], w3_d[:])
            nc.gpsimd.dma_start(wwT4[:], wwT4_d[:])
            h = MH // 2
            nc.sync.dma_start(x_m[:, 0:h], x_m_d[:, 0:h])
            nc.sync.dma_start(x_aug[:, 0:N // 4], x_aug_d[:, 0:N // 4])
            nc.sync.dma_start(x_aug[:, N // 4:N // 2],
                              x_aug_d[:, N // 4:N // 2])
            nc.sync.dma_start(x_m[:, h:MH], x_m_d[:, h:MH])
            for k in range(2, 4):
                s = slice(k * (N // 4), (k + 1) * (N // 4))
                nc.sync.dma_start(x_aug[:, s], x_aug_d[:, s])
            # resid DMAs are emitted later (inside the steady loop) so they
            # don't compete with the x DMAs for bandwidth at startup

            # ---- y accumulators: 13 chunks packed 4-per-bank; bank 3
            # columns 256.. are free (chunk 12 is 256 wide) and host the
            # per-block g^T projection scratch
            y_ps = [ypool.tile([128, YCH], F32, tag=f"y{t}", name=f"y{t}")
                    for t in range(4)]

            def y_slot(j):
                return y_ps[j // 4][32 * (j % 4):32 * (j % 4) + 32, :]

            # ---- projections ----
            # u (full n range, t-scaled cols included): psum borrows y
            # banks 0/1; copies alternate DVE/ACT (gpsimd can't read PSUM).
            def emit_proj_u(k):
                off, cw = U_CHUNKS[k]
                pu = y_ps[k % 2][0:2 * CI, 0:512]
                for t in range(2):
                    nc.tensor.matmul(pu[CI * t:CI * (t + 1), :cw], wuT,
                                     x_aug[:, off:off + cw],
                                     start=True, stop=True,
                                     tile_position=(0, CI * t))
                if k % 2 == 0:
                    nc.vector.tensor_copy(u_sb[:, off:off + cw], pu[:, :cw])
                else:
                    nc.scalar.copy(u_sb[:, off:off + cw], pu[:, :cw])

            # v (this core's m range): psum borrows y bank 2; copies on ACT
            # (they precede the exps in ACT's in-order queue).
            def emit_proj_v(k):
                off, cw = V_CHUNKS[k]
                pv = y_ps[2][0:2 * CI, 0:512]
                for t in range(2):
                    nc.tensor.matmul(pv[CI * t:CI * (t + 1), :cw], wvT,
                                     x_m[:, off:off + cw],
                                     start=True, stop=True,
                                     tile_position=(0, CI * t))
                nc.scalar.copy(v_sb[:, off:off + cw], pv[:, :cw])

            # g^T projection: small matmuls borrowing the y banks (all
            # strictly before the first y matmul).  gt_sb is laid out
            # bank-major so each bank evacuates with ONE wide DVE copy.
            GNB = [7, 6, 6, 6]
            GBASE = [0, 7, 13, 19]

            def gcol(i):
                return CI * (GBASE[i % 4] + i // 4)

            def emit_proj_g(i):
                pg = y_ps[i % 4][:, CI * (i // 4):CI * (i // 4 + 1)]
                nc.tensor.matmul(pg, x_m[:, i * MB:(i + 1) * MB], wgT,
                                 start=True, stop=True,
                                 skip_group_check=True)

            def emit_g_copies():
                for b in range(4):
                    n = GNB[b]
                    nc.vector.tensor_copy(
                        gt_sb[:, CI * GBASE[b]:CI * (GBASE[b] + n)],
                        y_ps[b][:, 0:CI * n])

            gts_prev = None
            exp_prev = None

            with (
                tc.tile_pool(name="spsumA", bufs=1, space="PSUM") as spoolA,
                tc.tile_pool(name="spsumB", bufs=1, space="PSUM") as spoolB,
            ):
                mm_state = [0]

                def emit_s_chunk_mms(i, ci, sp):
                    off, cw = S_CHUNKS[ci]
                    for s2 in range(0, cw, 512):
                        w2 = min(512, cw - s2)
                        g = CI * (mm_state[0] % 2)  # alternate row groups
                        mm_state[0] += 1
                        nc.tensor.matmul(
                            sp[:, s2:s2 + w2],
                            v_sb[g:g + CI, i * MB:(i + 1) * MB],
                            u_sb[g:g + CI, off + s2:off + s2 + w2],
                            start=True, stop=True)

                def emit_consumer(ci, sp, exp_t, sums):
                    off, cw = S_CHUNKS[ci]
                    if ci in DVE_CHUNKS:
                        nc.vector.tensor_scalar(
                            out=exp_t[:, off:off + cw].bitcast(I16),
                            in0=sp[:, :cw], scalar1=128.0,
                            scalar2=float(C_SCHRAUD), op0=MULT, op1=ADD)
                        if ci == 2:
                            # one reduce covers chunks 1+2 (cols 1024:3072)
                            nc.vector.tensor_reduce(sums[:, 4:5],
                                                    exp_t[:, 1024:3072],
                                                    AXLX, ADD)
                        elif ci == 6:
                            nc.vector.tensor_reduce(sums[:, 5:6],
                                                    exp_t[:, off:off + cw],
                                                    AXLX, ADD)
                    else:
                        sc = ACT_SCOL[ci]
                        nc.scalar.activation(
                            exp_t[:, off:off + cw], sp[:, :cw], EXP,
                            accum_out=sums[:, sc:sc + 1])

                def emit_merge(i, sums):
                    tot = wpool.tile([128, 1], F32, tag="tot", name="tot")
                    nc.vector.tensor_reduce(tot[:], sums[:, 0:6], AXLX, ADD)
                    rec = wpool.tile([128, 1], F32, tag="rec", name="rec")
                    nc.vector.reciprocal(rec[:], tot[:])
                    gts = wpool.tile([128, CI], BF16, tag="gts", name="gts")
                    nc.vector.tensor_scalar_mul(
                        gts[:], gt_sb[:, gcol(i):gcol(i) + CI], rec[:])
                    return gts

                def emit_y_mms(iy, js, gts):
                    for j in js:
                        off, cw = Y_CHUNKS[j]
                        nc.tensor.matmul(
                            y_slot(j)[:, :cw], gts,
                            exp_prev[:, off:off + cw],
                            start=(iy == 0), stop=(iy == NBLK - 1),
                            tile_position=(0, 32 * (j % 4)),
                            skip_group_check=True)

                def s_slot(ci):
                    pool = spoolA if ci % 2 == 0 else spoolB
                    return pool.tile([128, SCH], F32, tag="s", name="sp")

                # ---- block 0: projections interleaved with its S chunks
                exp0 = dpool.tile([128, N], BF16, tag="expS", name="exp_t")
                sums0 = wpool.tile([128, NSC + 1], F32, tag="sums",
                                   name="sums")
                emit_proj_v(0)
                emit_proj_v(1)
                emit_proj_v(2)
                for ci in range(NSC):
                    for k in range(2 * ci, min(2 * ci + 2, len(U_CHUNKS))):
                        emit_proj_u(k)
                    sp = s_slot(ci)
                    emit_s_chunk_mms(0, ci, sp)
                    emit_consumer(ci, sp, exp0, sums0)
                    if ci == 1:
                        emit_proj_v(3)
                        emit_proj_v(4)
                    elif ci == 2:
                        emit_proj_v(5)
                        emit_proj_v(6)
                for gi in range(NBLK):
                    emit_proj_g(gi)
                emit_g_copies()
                exp_prev = exp0
                gts_prev = emit_merge(0, sums0)[:]

                # ---- steady-state blocks ----
                for i in range(1, NBLK):
                    exp_t = dpool.tile([128, N], BF16, tag="expS",
                                       name="exp_t")
                    sums = wpool.tile([128, NSC + 1], F32, tag="sums",
                                      name="sums")
                    for ci in range(NSC):
                        sp = s_slot(ci)
                        emit_s_chunk_mms(i, ci, sp)
                        emit_consumer(ci, sp, exp_t, sums)
                        emit_y_mms(i - 1, Y_SCHED[ci], gts_prev)
                    if i == 2:
                        for k in range(4):
                            s = slice(k * (N // 4), (k + 1) * (N // 4))
                            nc.sync.dma_start(resid[:, s], resid_d[:, s])
                    gts_prev = emit_merge(i, sums)[:]
                    exp_prev = exp_t

            # ---- tail: last block's y matmuls then the output projection
            # pipeline; copies and adds are spread over ACT/DVE/GPSIMD and
            # the output DMAs over the sync+gpsimd queues.
            # tail: last block's y matmuls as one burst (they pipeline on
            # the four column groups), then the output chains stream on
            # ACT (copies) / PE (proj) / DVE (adds) / two DMA queues.
            with tc.tile_pool(name="fpsum", bufs=2, space="PSUM") as fpool:
                i = NBLK - 1

                def emit_last_y(j):
                    off, cw = Y_CHUNKS[j]
                    nc.tensor.matmul(
                        y_slot(j)[:, :cw], gts_prev,
                        exp_prev[:, off:off + cw],
                        start=(i == 0), stop=True,
                        tile_position=(0, 32 * (j % 4)),
                        skip_group_check=True)

                # y-mm burst; after each bank's 4 chunks are done, ONE wide
                # ACT copy evacuates the whole bank (4 chunks at partition
                # offsets 0/32/64/96) to SBUF.
                bank_done = {3: 0, 7: 1, 11: 2, 12: 3}
                for j in range(len(Y_CHUNKS)):
                    emit_last_y(j)
                    if j in bank_done:
                        b = bank_done[j]
                        pr, w = (128, YCH) if b < 3 else (32, 256)
                        nc.scalar.copy(y_sbs[b][0:pr, 0:w],
                                       y_ps[b][0:pr, 0:w])
                # paired projections into [64, 1024] psum, wide adds, wide
                # output DMAs on two queues
                for k in range(7):
                    js = [j for j in (2 * k, 2 * k + 1) if j < len(Y_CHUNKS)]
                    fp = fpool.tile([C, 2 * YCH], F32, tag="f")
                    for j in js:
                        off, cw = Y_CHUNKS[j]
                        p = 32 * (j % 4)
                        nc.tensor.matmul(
                            fp[:, (j % 2) * YCH:(j % 2) * YCH + cw],
                            wwT4[p:p + 32, :], y_sbs[j // 4][p:p + 32, :cw],
                            start=True, stop=True, tile_position=(p, 0))
                    off0 = Y_CHUNKS[js[0]][0]
                    w = sum(Y_CHUNKS[j][1] for j in js)
                    ot = wpool.tile([C, 2 * YCH], F32, tag="ot")
                    nc.vector.tensor_add(
                        ot[:, :w], fp[:, :w], resid[:, off0:off0 + w])
                    deng = nc.sync if k % 2 == 0 else nc.gpsimd
                    deng.dma_start(out_d[:, off0:off0 + w], ot[:, :w])

    nc.compile()
    return nc


def make_in_maps(x, Wg, bg, Wu, bu, Wv, bv, Ww, bw):
    x = np.asarray(x, np.float32)
    x16f = x.astype(np.float16)
    ones = np.ones((1, N), np.float32)
    wuT = np.concatenate([np.asarray(Wu, np.float32).T,
                          np.asarray(bu, np.float32)[None, :]], 0)
    wvT = np.concatenate([np.asarray(Wv, np.float32).T,
                          np.asarray(bv, np.float32)[None, :]], 0)
    wgT = np.concatenate([np.asarray(Wg, np.float32).T,
                          np.asarray(bg, np.float32)[None, :]], 0)
    w3 = np.concatenate([wuT, wvT, wgT], 1).astype(np.float16)
    wwT4 = np.concatenate(
        [np.ascontiguousarray(np.asarray(Ww, np.float32).T)] * 4,
        0).astype(np.float16)
    bw = np.asarray(bw, np.float32)

    # columns of x_aug feeding DVE chunks carry the log2(e) scale so the
    # S matmul directly produces t = S*log2e there
    colscale = np.ones((1, N), np.float32)
    for ci in DVE_CHUNKS:
        off, cw = S_CHUNKS[ci]
        colscale[0, off:off + cw] = LOG2E

    in_maps = []
    for core in range(NCORES):
        b, hh = divmod(core, 2)
        xb = x[b].reshape(C, N)
        x_aug = (np.concatenate([xb, ones], 0) * colscale).astype(np.float16)
        x_m = np.ascontiguousarray(
            np.concatenate([x16f[b].reshape(C, N), ones.astype(np.float16)],
                           0)[:, hh * MH:(hh + 1) * MH])
        if hh == 1:
            residc = xb + bw[:, None]
        else:
            residc = np.zeros((C, N), np.float32)
        in_maps.append({
            "x_aug": np.ascontiguousarray(x_aug),
            "x_m": x_m,
            "w3": np.ascontiguousarray(w3),
            "wwT4": np.ascontiguousarray(wwT4),
            "resid": np.ascontiguousarray(residc),
        })
    return in_maps


_NC = None


def kernel(x, Wg, bg, Wu, bu, Wv, bv, Ww, bw, _trace=False):
    global _NC
    if _NC is None:
        _NC = build_nc()
    in_maps = make_in_maps(x, Wg, bg, Wu, bu, Wv, bv, Ww, bw)
    res = run_bass_kernel_spmd(_NC, in_maps, list(range(NCORES)), trace=_trace)
    outs = [r["out"] for r in res.results]
    full = np.empty((B, C, H, W), np.float32)
    for b in range(B):
        full[b] = (outs[2 * b] + outs[2 * b + 1]).reshape(C, H, W)
    kernel.last_results = res
    return full


if __name__ == "__main__":
    rng = np.random.default_rng(0)
    s_in, s_mid = 1.0 / np.sqrt(C), 1.0 / np.sqrt(CI)
    ins = dict(
        x=rng.standard_normal((B, C, H, W), np.float32),
        Wg=(rng.standard_normal((CI, C)) * s_in).astype(np.float32),
        bg=(rng.standard_normal(CI) * 0.01).astype(np.float32),
        Wu=(rng.standard_normal((CI, C)) * s_in).astype(np.float32),
        bu=(rng.standard_normal(CI) * 0.01).astype(np.float32),
        Wv=(rng.standard_normal((CI, C)) * s_in).astype(np.float32),
        bv=(rng.standard_normal(CI) * 0.01).astype(np.float32),
        Ww=(rng.standard_normal((C, CI)) * s_mid).astype(np.float32),
        bw=(rng.standard_normal(C) * 0.01).astype(np.float32),
    )
    out = kernel(**ins)
    print("kernel output", out.shape, out.dtype)


# revision 51
# speedup vs baseline: 1.2119x; 1.2119x over previous
# Trainium2 Bass kernel for NonLocalBlock (B=4, C=64, CI=32, H=W=80).
#
# Math (per batch, N = H*W = 6400):
#   u = Wu@x+bu, v = Wv@x+bv, g = Wg@x+bg           [CI, N]
#   f[n,m] = sum_c u[c,n] v[c,m]; softmax over n (axis=1 of f)
#   y[c,n] = sum_m f_sm[n,m] g[c,m];  out = Ww@y + bw + x
#
# Define S = v^T u  (S[m,n] = f[n,m]).  The softmax axis n is the FREE
# axis of S rows, so processing S in 128-row blocks makes the softmax
# fully row-local.  y = g @ softmax_rows(S).
#
# Sharding: 8 cores = 4 batches x 2 halves of the m axis.  Each core
# computes a partial y (sum over its 3200 m rows), applies the output
# projection, and the host adds the two halves (bias+residual carried
# by the odd core via `resid`; even cores get zeros).
#
# Exp is split across engines: ACT chunks {0,2,4,6} of each 128x6400
# block run the exact Exp activation (rowsums via fused accum); DVE
# chunks {1,3,5} use the Schraudolph bit trick - the host pre-scales
# the matching x_aug columns by log2(e) so the S matmul directly
# yields t = S*log2e, and one DVE tensor_scalar computes
# i16 = round(t*128 + 16248.75), whose bits reinterpreted as bf16 are
# 2^t = e^S with ~1.8% rms multiplicative error (zero mean).  Softmax
# normalization + averaging over m wash this to ~1e-3 in the output.
# DVE-chunk rowsums come from bf16 tensor_reduce on DVE/GPSIMD.
#
# Engine budget per core/block (steady state): PE ~5.4-7us (13 S
# matmuls + 13 y matmuls, fp16/bf16 at 1 col/cycle), ACT ~5.0us,
# DVE ~4.6us, GPSIMD ~3.5us.  PE is the roofline.

import numpy as np

import concourse.bass as bass
import concourse.mybir as mybir
from concourse import bacc, tile
from concourse.bass_utils import run_bass_kernel_spmd

F32 = mybir.dt.float32
BF16 = mybir.dt.bfloat16
F16 = mybir.dt.float16
I16 = mybir.dt.int16

B, C, CI, H, W = 4, 64, 32, 80, 80
N = H * W              # 6400
NCORES = 8
MH = N // 2            # 3200 rows of S per core
MB = 128               # S row-block
NBLK = MH // MB        # 25 blocks per core
SCH = 1024             # S free-dim chunk held in PSUM (2 banks)
YCH = 512              # y free-dim chunk (1 bank)

EXP = mybir.ActivationFunctionType.Exp
ADD = mybir.AluOpType.add
MULT = mybir.AluOpType.mult
AXLX = mybir.AxisListType.X

LOG2E = 1.4426950408889634
C_SCHRAUD = 16248.75   # 127*128 - 128*log2(mean mult err), zero-mean tuned

# chunk assignment within each block (indices into S_CHUNKS); sized by
# measured rates: ACT ~1.37 ns/elem (exp+accum), DVE ~1.97 ns/elem
# (schraudolph + separate rowsum reduce).  Consumers must ALTERNATE
# between ACT and DVE chunk-by-chunk or the spool WAR chain serializes.
DVE_CHUNKS = (1, 3, 6)          # schraudolph on DVE (host scales these cols)


def _ceil_chunks(total, step):
    out = []
    off = 0
    while off < total:
        out.append((off, min(step, total - off)))
        off += step
    return out


S_CHUNKS = _ceil_chunks(N, SCH)      # 6 x 1024 + 256
Y_CHUNKS = _ceil_chunks(N, YCH)      # 12 x 512 + 256
U_CHUNKS = _ceil_chunks(N, 512)      # 12 x 512 + 256
V_CHUNKS = _ceil_chunks(MH, 512)     # 6 x 512 + 128
NSC = len(S_CHUNKS)                  # 7
# y chunks interleaved into the S chunk stream: after S chunk ci of
# block i, emit y chunks Y_SCHED[ci] of block i-1.  Quads use all four
# PE column groups so the matmuls can pipeline on disjoint tiles.
Y_SCHED = [(0, 1), (2, 3), (4, 5), (6, 7), (8, 9), (10, 11), (12,)]


def build_nc():
    nc = bacc.Bacc("TRN2", target_bir_lowering=False, debug=False,
                   num_devices=NCORES)

    x_aug_d = nc.dram_tensor("x_aug", [C + 1, N], F16, kind="ExternalInput")
    x_m_d = nc.dram_tensor("x_m", [C + 1, MH], F16, kind="ExternalInput")
    w3_d = nc.dram_tensor("w3", [C + 1, 3 * CI], F16, kind="ExternalInput")
    wwT4_d = nc.dram_tensor("wwT4", [128, C], F16, kind="ExternalInput")
    resid_d = nc.dram_tensor("resid", [C, N], F32, kind="ExternalInput")
    out_d = nc.dram_tensor("out", [C, N], F32, kind="ExternalOutput")

    with tile.TileContext(nc) as tc:
        with (
            tc.tile_pool(name="const", bufs=1) as cpool,
            tc.tile_pool(name="big", bufs=2) as dpool,
            tc.tile_pool(name="small", bufs=3) as wpool,
            tc.tile_pool(name="ypsum", bufs=1, space="PSUM") as ypool,
        ):
            # ---- persistent SBUF tiles ----
            x_aug = cpool.tile([C + 1, N], F16, tag="xa")
            x_m = cpool.tile([C + 1, MH], F16, tag="xm")
            u_sb = cpool.tile([2 * CI, N], F16, tag="u")     # 2 row groups
            v_sb = cpool.tile([2 * CI, MH], F16, tag="v")
            gt_sb = cpool.tile([128, NBLK * CI], F32, tag="gt")
            w3 = cpool.tile([C + 1, 3 * CI], F16, tag="w3")
            wwT4 = cpool.tile([128, C], F16, tag="ww")
            resid = cpool.tile([C, N], F32, tag="resid")
            # tail staging: one [128, 512] image per y PSUM bank (each bank
            # holds four y chunks at partition offsets 0/32/64/96)
            y_sbs = [cpool.tile([128, YCH], F16, tag=f"ysb{t}",
                                name=f"ysb{t}") for t in range(4)]
            wuT = w3[:, 0:CI]
            wvT = w3[:, CI:2 * CI]
            wgT = w3[:, 2 * CI:3 * CI]

            # ---- input DMAs: sync carries the x tensors (ordered so the
            # first projections can start asap), gpsimd the small weights
            nc.gpsimd.dma_start(w3[:], w3_d[:])
            nc.gpsimd.dma_start(wwT4[:], wwT4_d[:])
            h = MH // 2
            nc.sync.dma_start(x_m[:, 0:h], x_m_d[:, 0:h])
            nc.sync.dma_start(x_aug[:, 0:N // 4], x_aug_d[:, 0:N // 4])
            nc.sync.dma_start(x_aug[:, N // 4:N // 2],
                              x_aug_d[:, N // 4:N // 2])
            nc.sync.dma_start(x_m[:, h:MH], x_m_d[:, h:MH])
            for k in range(2, 4):
                s = slice(k * (N // 4), (k + 1) * (N // 4))
                nc.sync.dma_start(x_aug[:, s], x_aug_d[:, s])
            # resid DMAs are emitted later (inside the steady loop) so they
            # don't compete with the x DMAs for bandwidth at startup

            # ---- y accumulators: 13 chunks packed 4-per-bank; bank 3
            # columns 256.. are free (chunk 12 is 256 wide) and host the
            # per-block g^T projection scratch
            y_ps = [ypool.tile([128, YCH], F32, tag=f"y{t}", name=f"y{t}")
                    for t in range(4)]

            def y_slot(j):
                return y_ps[j // 4][32 * (j % 4):32 * (j % 4) + 32, :]

            # ---- projections ----
            # u (full n range, t-scaled cols included): psum borrows y
            # banks 0/1; copies alternate DVE/ACT (gpsimd can't read PSUM).
            def emit_proj_u(k):
                off, cw = U_CHUNKS[k]
                pu = y_ps[k % 2][0:2 * CI, 0:512]
                for t in range(2):
                    nc.tensor.matmul(pu[CI * t:CI * (t + 1), :cw], wuT,
                                     x_aug[:, off:off + cw],
                                     start=True, stop=True,
                                     tile_position=(0, CI * t))
                if k % 2 == 0:
                    nc.vector.tensor_copy(u_sb[:, off:off + cw], pu[:, :cw])
                else:
                    nc.scalar.copy(u_sb[:, off:off + cw], pu[:, :cw])

            # v (this core's m range): psum borrows y bank 2; copies on ACT
            # (they precede the exps in ACT's in-order queue).
            def emit_proj_v(k):
                off, cw = V_CHUNKS[k]
                pv = y_ps[2][0:2 * CI, 0:512]
                for t in range(2):
                    nc.tensor.matmul(pv[CI * t:CI * (t + 1), :cw], wvT,
                                     x_m[:, off:off + cw],
                                     start=True, stop=True,
                                     tile_position=(0, CI * t))
                nc.scalar.copy(v_sb[:, off:off + cw], pv[:, :cw])

            # g^T projection: small matmuls borrowing the y banks (all
            # strictly before the first y matmul).  gt_sb is laid out
            # bank-major so each bank evacuates with ONE wide DVE copy.
            GNB = [7, 6, 6, 6]
            GBASE = [0, 7, 13, 19]

            def gcol(i):
                return CI * (GBASE[i % 4] + i // 4)

            def emit_proj_g(i):
                pg = y_ps[i % 4][:, CI * (i // 4):CI * (i // 4 + 1)]
                nc.tensor.matmul(pg, x_m[:, i * MB:(i + 1) * MB], wgT,
                                 start=True, stop=True,
                                 skip_group_check=True)

            def emit_g_copies():
                for b in range(4):
                    n = GNB[b]
                    nc.vector.tensor_copy(
                        gt_sb[:, CI * GBASE[b]:CI * (GBASE[b] + n)],
                        y_ps[b][:, 0:CI * n])

            gts_prev = None
            exp_prev = None

            with (
                tc.tile_pool(name="spsumA", bufs=1, space="PSUM") as spoolA,
                tc.tile_pool(name="spsumB", bufs=1, space="PSUM") as spoolB,
            ):
                mm_state = [0]

                def emit_s_chunk_mms(i, ci, sp):
                    off, cw = S_CHUNKS[ci]
                    for s2 in range(0, cw, 512):
                        w2 = min(512, cw - s2)
                        g = CI * (mm_state[0] % 2)  # alternate row groups
                        mm_state[0] += 1
                        nc.tensor.matmul(
                            sp[:, s2:s2 + w2],
                            v_sb[g:g + CI, i * MB:(i + 1) * MB],
                            u_sb[g:g + CI, off + s2:off + s2 + w2],
                            start=True, stop=True)

                def emit_consumer(ci, sp, exp_t, sums):
                    off, cw = S_CHUNKS[ci]
                    if ci in DVE_CHUNKS:
                        nc.vector.tensor_scalar(
                            out=exp_t[:, off:off + cw].bitcast(I16),
                            in0=sp[:, :cw], scalar1=128.0,
                            scalar2=float(C_SCHRAUD), op0=MULT, op1=ADD)
                        nc.vector.tensor_reduce(sums[:, ci:ci + 1],
                                                exp_t[:, off:off + cw],
                                                AXLX, ADD)
                    else:
                        nc.scalar.activation(
                            exp_t[:, off:off + cw], sp[:, :cw], EXP,
                            accum_out=sums[:, ci:ci + 1])

                def emit_merge(i, sums):
                    tot = wpool.tile([128, 1], F32, tag="tot", name="tot")
                    nc.vector.tensor_reduce(tot[:], sums[:, 0:NSC], AXLX, ADD)
                    rec = wpool.tile([128, 1], F32, tag="rec", name="rec")
                    nc.vector.reciprocal(rec[:], tot[:])
                    gts = wpool.tile([128, CI], BF16, tag="gts", name="gts")
                    nc.vector.tensor_scalar_mul(
                        gts[:], gt_sb[:, gcol(i):gcol(i) + CI], rec[:])
                    return gts

                def emit_y_mms(iy, js, gts):
                    for j in js:
                        off, cw = Y_CHUNKS[j]
                        nc.tensor.matmul(
                            y_slot(j)[:, :cw], gts,
                            exp_prev[:, off:off + cw],
                            start=(iy == 0), stop=(iy == NBLK - 1),
                            tile_position=(0, 32 * (j % 4)),
                            skip_group_check=True)

                def s_slot(ci):
                    pool = spoolA if ci % 2 == 0 else spoolB
                    return pool.tile([128, SCH], F32, tag="s", name="sp")

                # ---- block 0: projections interleaved with its S chunks
                exp0 = dpool.tile([128, N], BF16, tag="expS", name="exp_t")
                sums0 = wpool.tile([128, NSC + 1], F32, tag="sums",
                                   name="sums")
                emit_proj_v(0)
                emit_proj_v(1)
                emit_proj_v(2)
                for ci in range(NSC):
                    for k in range(2 * ci, min(2 * ci + 2, len(U_CHUNKS))):
                        emit_proj_u(k)
                    sp = s_slot(ci)
                    emit_s_chunk_mms(0, ci, sp)
                    emit_consumer(ci, sp, exp0, sums0)
                    if ci == 1:
                        emit_proj_v(3)
                        emit_proj_v(4)
                    elif ci == 2:
                        emit_proj_v(5)
                        emit_proj_v(6)
                for gi in range(NBLK):
                    emit_proj_g(gi)
                emit_g_copies()
                exp_prev = exp0
                gts_prev = emit_merge(0, sums0)[:]

                # ---- steady-state blocks ----
                for i in range(1, NBLK):
                    exp_t = dpool.tile([128, N], BF16, tag="expS",
                                       name="exp_t")
                    sums = wpool.tile([128, NSC + 1], F32, tag="sums",
                                      name="sums")
                    for ci in range(NSC):
                        sp = s_slot(ci)
                        emit_s_chunk_mms(i, ci, sp)
                        emit_consumer(ci, sp, exp_t, sums)
                        emit_y_mms(i - 1, Y_SCHED[ci], gts_prev)
                    if i == 2:
                        for k in range(4):
                            s = slice(k * (N // 4), (k + 1) * (N // 4))
                            nc.sync.dma_start(resid[:, s], resid_d[:, s])
                    gts_prev = emit_merge(i, sums)[:]
                    exp_prev = exp_t

            # ---- tail: last block's y matmuls then the output projection
            # pipeline; copies and adds are spread over ACT/DVE/GPSIMD and
            # the output DMAs over the sync+gpsimd queues.
            # tail: last block's y matmuls as one burst (they pipeline on
            # the four column groups), then the output chains stream on
            # ACT (copies) / PE (proj) / DVE (adds) / two DMA queues.
            with tc.tile_pool(name="fpsum", bufs=2, space="PSUM") as fpool:
                i = NBLK - 1

                def emit_last_y(j):
                    off, cw = Y_CHUNKS[j]
                    nc.tensor.matmul(
                        y_slot(j)[:, :cw], gts_prev,
                        exp_prev[:, off:off + cw],
                        start=(i == 0), stop=True,
                        tile_position=(0, 32 * (j % 4)),
                        skip_group_check=True)

                # y-mm burst; after each bank's 4 chunks are done, ONE wide
                # ACT copy evacuates the whole bank (4 chunks at partition
                # offsets 0/32/64/96) to SBUF.
                bank_done = {3: 0, 7: 1, 11: 2, 12: 3}
                for j in range(len(Y_CHUNKS)):
                    emit_last_y(j)
                    if j in bank_done:
                        b = bank_done[j]
                        pr, w = (128, YCH) if b < 3 else (32, 256)
                        nc.scalar.copy(y_sbs[b][0:pr, 0:w],
                                       y_ps[b][0:pr, 0:w])
                # paired projections into [64, 1024] psum, wide adds, wide
                # output DMAs on two queues
                for k in range(7):
                    js = [j for j in (2 * k, 2 * k + 1) if j < len(Y_CHUNKS)]
                    fp = fpool.tile([C, 2 * YCH], F32, tag="f")
                    for j in js:
                        off, cw = Y_CHUNKS[j]
                        p = 32 * (j % 4)
                        nc.tensor.matmul(
                            fp[:, (j % 2) * YCH:(j % 2) * YCH + cw],
                            wwT4[p:p + 32, :], y_sbs[j // 4][p:p + 32, :cw],
                            start=True, stop=True, tile_position=(p, 0))
                    off0 = Y_CHUNKS[js[0]][0]
                    w = sum(Y_CHUNKS[j][1] for j in js)
                    ot = wpool.tile([C, 2 * YCH], F32, tag="ot")
                    nc.vector.tensor_add(
                        ot[:, :w], fp[:, :w], resid[:, off0:off0 + w])
                    deng = nc.sync if k % 2 == 0 else nc.gpsimd
                    deng.dma_start(out_d[:, off0:off0 + w], ot[:, :w])

    nc.compile()
    return nc


def make_in_maps(x, Wg, bg, Wu, bu, Wv, bv, Ww, bw):
    x = np.asarray(x, np.float32)
    x16f = x.astype(np.float16)
    ones = np.ones((1, N), np.float32)
    wuT = np.concatenate([np.asarray(Wu, np.float32).T,
                          np.asarray(bu, np.float32)[None, :]], 0)
    wvT = np.concatenate([np.asarray(Wv, np.float32).T,
                          np.asarray(bv, np.float32)[None, :]], 0)
    wgT = np.concatenate([np.asarray(Wg, np.float32).T,
                          np.asarray(bg, np.float32)[None, :]], 0)
    w3 = np.concatenate([wuT, wvT, wgT], 1).astype(np.float16)
    wwT4 = np.concatenate(
        [np.ascontiguousarray(np.asarray(Ww, np.float32).T)] * 4,
        0).astype(np.float16)
    bw = np.asarray(bw, np.float32)

    # columns of x_aug feeding DVE chunks carry the log2(e) scale so the
    # S matmul directly produces t = S*log2e there
    colscale = np.ones((1, N), np.float32)
    for ci in DVE_CHUNKS:
        off, cw = S_CHUNKS[ci]
        colscale[0, off:off + cw] = LOG2E

    in_maps = []
    for core in range(NCORES):
        b, hh = divmod(core, 2)
        xb = x[b].reshape(C, N)
        x_aug = (np.concatenate([xb, ones], 0) * colscale).astype(np.float16)
        x_m = np.ascontiguousarray(
            np.concatenate([x16f[b].reshape(C, N), ones.astype(np.float16)],
                           0)[:, hh * MH:(hh + 1) * MH])
        if hh == 1:
            residc = xb + bw[:, None]
        else:
            residc = np.zeros((C, N), np.float32)
        in_maps.append({
            "x_aug": np.ascontiguousarray(x_aug),
            "x_m": x_m,
            "w3": np.ascontiguousarray(w3),
            "wwT4": np.ascontiguousarray(wwT4),
            "resid": np.ascontiguousarray(residc),
        })
    return in_maps


_NC = None


def kernel(x, Wg, bg, Wu, bu, Wv, bv, Ww, bw, _trace=False):
    global _NC
    if _NC is None:
        _NC = build_nc()
    in_maps = make_in_maps(x, Wg, bg, Wu, bu, Wv, bv, Ww, bw)
    res = run_bass_kernel_spmd(_NC, in_maps, list(range(NCORES)), trace=_trace)
    outs = [r["out"] for r in res.results]
    full = np.empty((B, C, H, W), np.float32)
    for b in range(B):
        full[b] = (outs[2 * b] + outs[2 * b + 1]).reshape(C, H, W)
    kernel.last_results = res
    return full


if __name__ == "__main__":
    rng = np.random.default_rng(0)
    s_in, s_mid = 1.0 / np.sqrt(C), 1.0 / np.sqrt(CI)
    ins = dict(
        x=rng.standard_normal((B, C, H, W), np.float32),
        Wg=(rng.standard_normal((CI, C)) * s_in).astype(np.float32),
        bg=(rng.standard_normal(CI) * 0.01).astype(np.float32),
        Wu=(rng.standard_normal((CI, C)) * s_in).astype(np.float32),
        bu=(rng.standard_normal(CI) * 0.01).astype(np.float32),
        Wv=(rng.standard_normal((CI, C)) * s_in).astype(np.float32),
        bv=(rng.standard_normal(CI) * 0.01).astype(np.float32),
        Ww=(rng.standard_normal((C, CI)) * s_mid).astype(np.float32),
        bw=(rng.standard_normal(C) * 0.01).astype(np.float32),
    )
    out = kernel(**ins)
    print("kernel output", out.shape, out.dtype)
